# revision 2
# baseline (speedup 1.0000x reference)
"""Grouped MLP (MoE expert-parallel) Trainium2 kernel.

Problem: hidden_states [131072, 1024] f32, 8 experts each owning a contiguous
16384-token block; per expert: SwiGLU MLP with gate_up [1024, 1024] and
down [512, 1024].

Sharding: expert-parallel — core e computes expert e's token block entirely
locally (no collectives). Inputs are sliced host-side, outputs concatenated.

Per-core kernel (per 128-token tile):
  - load x tile [128, 1024] f32 (HWDGE)
  - PE-transpose 8x [128,128] -> xT (f32r, rounded during PSUM->SBUF copy)
  - mm1: PSUM[128t, 512f] x2 accumulating over 8 h-slices;
         lhsT = xT[:,k,:] (stationary), rhs = W1[k, f-chunk] (f32r, resident)
  - SwiGLU: silu(gate) on ACT, * up on DVE (f32)
  - PE-transpose 4x -> hT (f32r)
  - mm2: PSUM[128t, 512]x2 over 4 i-slices with W2 resident
  - copy PSUM -> SBUF f32 out tile, DMA store (natural [tokens, H] layout)

float32r gives full PE rate (1 cyc/row at N=512) at ~1.5e-4 relative error.
"""

import numpy as np

E = 8
H = 1024
I = 512
T_PER_CORE = 16384
N_CORES = 8

_cache = {}


def _build_nc(n_tiles):
    import concourse.mybir as mybir
    import concourse.tile as tile
    from concourse import bacc
    from concourse.masks import make_identity

    f32 = mybir.dt.float32
    f32r = mybir.dt.float32r

    nc = bacc.Bacc(None, target_bir_lowering=False)
    n_tok = n_tiles * 128
    x = nc.dram_tensor("x", [n_tok, H], f32, kind="ExternalInput")
    w1 = nc.dram_tensor("w1", [H, 2 * I], f32, kind="ExternalInput")
    w2 = nc.dram_tensor("w2", [I, H], f32, kind="ExternalInput")
    out = nc.dram_tensor("out", [n_tok, H], f32, kind="ExternalOutput")

    with tile.TileContext(nc) as tc:
        with (
            tc.tile_pool(name="const", bufs=1) as const,
            tc.tile_pool(name="xin", bufs=3) as xin,
            tc.tile_pool(name="xtp", bufs=2) as xtp,
            tc.tile_pool(name="actp", bufs=3) as actp,
            tc.tile_pool(name="htp", bufs=2) as htp,
            tc.tile_pool(name="outp", bufs=3) as outp,
            tc.tile_pool(name="tp_ps", bufs=2, space="PSUM") as tp_ps_pool,
            tc.tile_pool(name="mm1_ps", bufs=4, space="PSUM") as mm1_ps_pool,
            tc.tile_pool(name="mm2_ps", bufs=2, space="PSUM") as mm2_ps_pool,
        ):
            # Resident weights, rounded to f32r during the load DMA (SWDGE cast).
            w1_sb = const.tile([128, H // 128, 2 * I], f32r)
            nc.gpsimd.dma_start(w1_sb[:], w1.ap().rearrange("(ho p) f -> p ho f", p=128))
            w2_sb = const.tile([128, I // 128, H], f32r)
            nc.gpsimd.dma_start(w2_sb[:], w2.ap().rearrange("(io p) f -> p io f", p=128))
            ident = const.tile([128, 128], f32)
            make_identity(nc, ident)

            for t in range(n_tiles):
                x_t = xin.tile([128, H], f32, tag="x")
                nc.sync.dma_start(x_t[:], x.ap()[t * 128 : (t + 1) * 128, :])

                # transpose x tile -> xT [128, 8, 128] f32r
                xT = xtp.tile([128, H // 128, 128], f32r, tag="xT")
                for g in range(2):
                    tp_ps = tp_ps_pool.tile([128, 4, 128], f32, tag="tp")
                    for j in range(4):
                        k = g * 4 + j
                        nc.tensor.transpose(
                            tp_ps[:, j, :], x_t[:, k * 128 : (k + 1) * 128], ident
                        )
                    nc.vector.tensor_copy(xT[:, g * 4 : (g + 1) * 4, :], tp_ps[:])

                # mm1: two 512-wide feature chunks (gate, up)
                mm1_ps = []
                for f in range(2):
                    ps = mm1_ps_pool.tile([128, 512], f32, tag="mm1")
                    for k in range(H // 128):
                        nc.tensor.matmul(
                            ps[:],
                            xT[:, k, :],
                            w1_sb[:, k, f * 512 : (f + 1) * 512],
                            start=(k == 0),
                            stop=(k == H // 128 - 1),
                        )
                    mm1_ps.append(ps)

                # SwiGLU: h = gate * sigmoid(gate) * up
                s = actp.tile([128, 512], f32, tag="s")
                nc.scalar.activation(
                    s[:], mm1_ps[0][:], mybir.ActivationFunctionType.Sigmoid
                )
                t1 = actp.tile([128, 512], f32, tag="t1")
                nc.vector.tensor_mul(t1[:], s[:], mm1_ps[1][:])
                h = actp.tile([128, 512], f32, tag="h")
                nc.vector.tensor_mul(h[:], t1[:], mm1_ps[0][:])

                # transpose h -> hT [128, 4, 128] f32r
                hT = htp.tile([128, I // 128, 128], f32r, tag="hT")
                tp_ps = tp_ps_pool.tile([128, 4, 128], f32, tag="tp")
                for k in range(4):
                    nc.tensor.transpose(
                        tp_ps[:, k, :], h[:, k * 128 : (k + 1) * 128], ident
                    )
                nc.vector.tensor_copy(hT[:], tp_ps[:])

                # mm2 + output copy
                o_t = outp.tile([128, H], f32, tag="o")
                for f in range(2):
                    ps2 = mm2_ps_pool.tile([128, 512], f32, tag="mm2")
                    for k in range(I // 128):
                        nc.tensor.matmul(
                            ps2[:],
                            hT[:, k, :],
                            w2_sb[:, k, f * 512 : (f + 1) * 512],
                            start=(k == 0),
                            stop=(k == I // 128 - 1),
                        )
                    if f == 0:
                        nc.scalar.copy(o_t[:, 0:512], ps2[:])
                    else:
                        nc.vector.tensor_copy(o_t[:, 512:1024], ps2[:])

                nc.sync.dma_start(out.ap()[t * 128 : (t + 1) * 128, :], o_t[:])

    nc.compile()
    return nc


def _get_nc(n_tiles):
    if n_tiles not in _cache:
        _cache[n_tiles] = _build_nc(n_tiles)
    return _cache[n_tiles]


def kernel(hidden_states, gate_up_proj, down_proj, num_tokens_per_expert):
    sizes = np.asarray(num_tokens_per_expert)
    offsets = np.concatenate([[0], np.cumsum(sizes)])
    uniform = (
        sizes.shape[0] == E
        and np.all(sizes == T_PER_CORE)
        and hidden_states.shape == (E * T_PER_CORE, H)
    )
    if not uniform:
        # Fallback: host-side numpy (routing metadata other than the
        # compiled uniform case).
        outs = []
        for e in range(sizes.shape[0]):
            xe = hidden_states[offsets[e] : offsets[e + 1]].astype(np.float32)
            merged = xe @ gate_up_proj[e]
            gate, up = merged[:, :I], merged[:, I:]
            he = (gate / (1.0 + np.exp(-gate))) * up
            outs.append(he @ down_proj[e])
        return np.concatenate(outs, axis=0).astype(hidden_states.dtype)

    from concourse.bass_utils import run_bass_kernel_spmd

    nc = _get_nc(T_PER_CORE // 128)
    hs = np.ascontiguousarray(np.asarray(hidden_states, dtype=np.float32))
    w1 = np.ascontiguousarray(np.asarray(gate_up_proj, dtype=np.float32))
    w2 = np.ascontiguousarray(np.asarray(down_proj, dtype=np.float32))
    in_maps = [
        {
            "x": hs[e * T_PER_CORE : (e + 1) * T_PER_CORE],
            "w1": w1[e],
            "w2": w2[e],
        }
        for e in range(N_CORES)
    ]
    res = run_bass_kernel_spmd(nc, in_maps, core_ids=list(range(N_CORES)))
    return np.concatenate([r["out"] for r in res.results], axis=0)
